# revision 66
# baseline (speedup 1.0000x reference)
"""Trainium2 Bass kernel for multi-head attention (B=2, S=2048, D=1024, H=16, causal, RoPE).

Sharding: tensor-parallel over heads. Each of the 8 cores computes 2 heads
(128 of the 1024 q/k/v dims): QKV projections for its head slice, RoPE,
causal attention, and a partial output projection against its 128-column
slice of o_weight. The host sums the 8 partial outputs (the all-reduce).

Device-side layout choices:
  - QKV projections run as fp8e4 DoubleRow matmuls (2 contraction rows per
    partition, half cost per the PE model): x and the weights arrive as
    host-prepared fp8 hi+lo pairs (weights pre-scaled by 32 so fp8 normals
    cover them; the 32*32 factor is folded into the exp scale and into wo),
    and hi*hi + hi*lo + lo*hi compensation keeps the error ~0.1%.
  - Activations live transposed: q/k are [128 (head dims), seq] so the
    scores matmul contracts dh on partitions. RoPE pairs are de-interleaved
    on the host (weight-row permutation) so pair partners sit 32 partitions
    apart; the rotate step is a single 128x128 sign-swap matmul (sperm).
  - Scores also run as fp8e4 DoubleRow at half cost: the stationary pair is
    (fp8(k), k - fp8(k)) so the k-side quantization error cancels; the
    moving q8 is a stride-0 broadcast over the pair dim. Scores are
    computed transposed ([sk, sq]) so P = exp(scores) feeds the PV matmul
    directly as the moving bf16 operand. V carries a block of 64 ones
    columns so the PV matmul also emits the softmax denominator;
    normalization is a plain reciprocal+multiply.
  - V is projected directly into [seq, dh] layout by using the (transposed)
    x tiles as the stationary operand - no on-chip transposes needed.
  - Work is software-pipelined: projection chunks run one chunk ahead of
    attention, output projections trail one chunk behind, both threaded
    through the attention tile loop. Dummy matmuls at t=0 cover the first
    DMAs and pre-ramp the PE p-state clock. Engine assignment of the
    PSUM-evacuation copies (ACT/DVE only - GPSIMD cannot touch PSUM) and
    the scheduling knobs below were tuned by timeline-simulator sweep.
  - Only one PSUM accumulation group may live per 2KB bank (zero region),
    so accumulators are bank-aligned and column-split starts are avoided.
  - Partial outputs are written in bf16 (summed in fp32 on the host).
"""

import numpy as np

D_MODEL = 1024
N_HEADS = 16
D_HEAD = 64
THETA = 10000.0
B = 2
S = 2048
N_CORES = 8
BS = B * S  # 4096
NQ = 512    # query chunk width
NK = 128    # key tile width

GROUP_TAIL = False
FINE_TAIL = False
N_DUMMY = 13
VPROJ_SPLIT = False
YS_MODE = 1
PV_DEPTH_N = 3
MASK_SPLIT = False
TMPQ_DVE = False    # q-proj PSUM->SBUF copy on DVE (else Pool)
K8_SWAP = False     # k8 copy on DVE + kres on Pool
PLOAD_V2 = False    # trig before x_lo in chunk loads
KRES = True         # k-side scores-stage residual compensation
KXLO = True         # k-projection x_lo term
# oproj mid quota per chunk (0,0)..(1,3); leftovers drain after the loop
OPROJ_Q = [0, 2, 2, 10, 0, 0, 4, 14]
OPROJ_PA = True     # final-chunk oproj uses the idle pa PSUM ring
OPROJ_PA_ACT = False  # tail ys copies fully on ACT
TAIL_CB = True      # interleave final-chunk oproj with per-tile normalize
ROPE_EARLY_DVE = True  # early chunks: rope k-mul on DVE to unload Pool
ROPE_EARLY_U = 2
WDMA_LATE = False   # wk/wv/m128 DMAs dispatch after chunk-0 x_hi
TMP_ENG = {"q": "dve", "k": "act"}  # per-proj tmp-copy engine
OPROJ_HALF = True
OPROJ_DMA_SPLIT = False  # tail y DMAs in halves (worse: dispatch overhead)
FAST_RECIP = False  # 5x faster approx reciprocal for softmax denominators
OPROJ_HALF2 = False  # half-granularity mids also for the (0,3) chunk
MERGE_V2 = False    # spread filler evenly across the chunk's passes
NORM_POOL = False   # normalize multiplies on Pool instead of DVE
PP_BUFS = 4
PYS_BUFS = 4
PX_BUFS = 3
PTMP_BUFS = 3

_RT = {}


def _build():
    if _RT:
        return _RT
    import sys
    try:
        import concourse.bass  # noqa: F401
    except ImportError:
        sys.path.insert(0, "/opt/trn_rl_repo")
    import concourse.mybir as mybir
    import concourse.tile as tile
    from concourse import bacc
    from concourse._compat import axon_active
    from concourse.bass_utils import run_bass_kernel_spmd

    f32 = mybir.dt.float32
    f32r = mybir.dt.float32r
    bf16 = mybir.dt.bfloat16
    fp8 = mybir.dt.float8e4
    EXP = mybir.ActivationFunctionType.Exp
    DR = mybir.MatmulPerfMode.DoubleRow

    nc = bacc.Bacc(
        "TRN2", target_bir_lowering=False, debug=not axon_active(),
        num_devices=N_CORES,
    )

    # x and the qkv weights arrive as fp8 hi/lo pairs (weights pre-scaled by
    # 32 on the host so fp8 normals cover them; the 32*32 factor is folded
    # into the exp scale and wo). Same DMA bytes as bf16.
    xh8 = nc.dram_tensor("xh8", [D_MODEL, BS], fp8, kind="ExternalInput").ap()
    xl8 = nc.dram_tensor("xl8", [D_MODEL, BS], fp8, kind="ExternalInput").ap()
    wq8 = nc.dram_tensor("wq8", [128, 2, 4, 2, 128], fp8, kind="ExternalInput").ap()
    wk8 = nc.dram_tensor("wk8", [128, 2, 4, 2, 128], fp8, kind="ExternalInput").ap()
    wv8 = nc.dram_tensor("wv8", [128, 2, 4, 2, 128], fp8, kind="ExternalInput").ap()
    wo = nc.dram_tensor("wo", [128, D_MODEL], f32r, kind="ExternalInput").ap()
    trig = nc.dram_tensor("trig", [128, 2, S], f32, kind="ExternalInput").ap()
    sperm = nc.dram_tensor("sperm", [128, 128], f32r, kind="ExternalInput").ap()
    mask128 = nc.dram_tensor("mask128", [128, 128], bf16, kind="ExternalInput").ap()
    y = nc.dram_tensor("y", [BS, D_MODEL], bf16, kind="ExternalOutput").ap()

    with tile.TileContext(nc) as tc:
        with (
            tc.tile_pool(name="singles", bufs=1) as singles,
            tc.tile_pool(name="px", bufs=PX_BUFS) as px,
            tc.tile_pool(name="ptmp", bufs=PTMP_BUFS) as ptmp,
            tc.tile_pool(name="pp", bufs=PP_BUFS) as pp,
            tc.tile_pool(name="pys", bufs=PYS_BUFS) as pys,
            tc.tile_pool(name="pr", bufs=3) as pr,
            tc.tile_pool(name="ps_a", bufs=2, space="PSUM") as ps_a,
            tc.tile_pool(name="ps_s", bufs=2, space="PSUM") as ps_s,
            tc.tile_pool(name="ps_o", bufs=2, space="PSUM") as ps_o,
        ):
            wq_sb = singles.tile([128, 2, 4, 2, 128], fp8, tag="wq")
            wk_sb = singles.tile([128, 2, 4, 2, 128], fp8, tag="wk")
            wv_sb = singles.tile([128, 2, 4, 2, 128], fp8, tag="wv")
            wo_sb = singles.tile([128, D_MODEL], f32r, tag="wo")
            sperm_sb = singles.tile([128, 128], f32r, tag="sperm")
            m128_sb = singles.tile([128, 128], bf16, tag="m128")
            warm_sb = singles.tile([1, 2], f32, tag="warm")
            # q8: rope'd q in fp8, one slot per chunk-within-batch.
            q8_sb = singles.tile([128, 4, NQ], fp8, tag="q8")
            # k8: [head dims, batch parity, (fp8(k), k-fp8(k)), key pos]
            k8_sb = singles.tile([128, 2, 2, S], fp8, tag="k8")
            oT_sb = singles.tile([128, BS], f32r, tag="oT")
            # V tiles: [seq-tile partitions, 32 tiles, 192]: cols 0:64 head A,
            # 64:128 ones, 128:192 head B. Head A lhsT = cols 0:128, head B
            # lhsT = cols 64:192; the ones block replicates the denominator.
            v_sb = singles.tile([128, 32, 192], bf16, tag="v")

            junk_sb = singles.tile([128, 512], f32r, tag="junk")

            nc.scalar.dma_start(out=wq_sb, in_=wq8)
            if not WDMA_LATE:
                nc.scalar.dma_start(out=wk_sb, in_=wk8)
            nc.scalar.dma_start(out=sperm_sb, in_=sperm)
            if not WDMA_LATE:
                nc.scalar.dma_start(out=wv_sb, in_=wv8)
                nc.scalar.dma_start(out=m128_sb, in_=mask128)
            nc.vector.memset(v_sb[:, :, 64:128].bitcast(bf16), 1.0)

            if not KRES:
                nc.vector.memset(k8_sb[:, :, 1, :], 0.0)

            # warm the ACT exp table before the first real exp
            nc.vector.memset(warm_sb, 0.0)
            nc.scalar.activation(warm_sb[:, 0:1], warm_sb[:, 1:2], EXP)

            # dummy matmuls: keep PE busy during the initial DMAs and ramp
            # the p-state clock before the first real matmul arrives
            nc.gpsimd.memset(junk_sb.bitcast(f32), 0.0)
            for w in range(N_DUMMY):
                jp = ps_s.tile([128, 2, 512], f32, tag="sps")
                nc.tensor.matmul(jp[:, 0, :], junk_sb[:, 0:128], junk_sb,
                                 start=True, stop=True)

            def RECIP(out, in_):
                if FAST_RECIP:
                    nc.vector.reciprocal_approx_fast(out, in_)
                else:
                    nc.vector.reciprocal(out, in_)

            def late_consts():
                if WDMA_LATE:
                    nc.scalar.dma_start(out=wv_sb, in_=wv8)
                    nc.scalar.dma_start(out=m128_sb, in_=mask128)
                nc.sync.dma_start(out=wo_sb, in_=wo)

            def proj_pieces(b, c, after_xt=None):
                """QKV projections + rope + V transpose for seq chunk c of
                batch b (512 positions), as a list of closures that can be
                threaded through the attention tile loop."""
                u = 4 * b + c
                s0 = 512 * c
                xt = []
                tmps = {}
                tg = [None]

                def p_load():
                    # q/k projections need only x_hi, so trig loads before x_lo
                    xt.extend([None] * 4)

                    def load(lo, h):
                        xth = px.tile([128, 4, 512], fp8, tag=f"xt{lo}{h}",
                                      name="xth")
                        nc.sync.dma_start(
                            out=xth,
                            in_=(xh8 if lo == 0 else xl8)
                            [512 * h:512 * (h + 1), 512 * u:512 * (u + 1)]
                            .rearrange("(a p) n -> p a n", p=128),
                        )
                        xt[2 * lo + h] = xth
                    load(0, 0)
                    load(0, 1)
                    if WDMA_LATE and u == 0:
                        # wk dispatches after chunk-0 x_hi, before trig/x_lo
                        nc.scalar.dma_start(out=wk_sb, in_=wk8)
                    if PLOAD_V2:
                        tg[0] = ptmp.tile([128, 2, 512], f32, tag="tg",
                                          name="tg")
                        nc.sync.dma_start(out=tg[0],
                                          in_=trig[:, :, s0:s0 + 512])
                        load(1, 0)
                        load(1, 1)
                    else:
                        load(1, 0)
                        load(1, 1)
                        tg[0] = ptmp.tile([128, 2, 512], f32, tag="tg",
                                          name="tg")
                        nc.sync.dma_start(out=tg[0],
                                          in_=trig[:, :, s0:s0 + 512])
                    if after_xt is not None:
                        after_xt()

                def xpair(lo, j):
                    # [128, 2, 512] fp8: K-chunk pair (2j, 2j+1)
                    h, a = divmod(2 * j, 4)
                    return xt[2 * lo + h][:, a:a + 2, :]

                def p_proj(w_sb, name):
                    # DoubleRow fp8, hi/lo compensated:
                    # q/k = x_hi*w_hi + x_hi*w_lo + x_lo*w_hi
                    def f():
                        ps = ps_a.tile([128, 512], f32, tag="pa")
                        mm = [(0, 0, j) for j in range(4)] + \
                             [(1, 0, j) for j in range(4)]
                        if name == "q" or KXLO:
                            mm += [(0, 1, j) for j in range(4)]
                        for n, (wl, xl, j) in enumerate(mm):
                            nc.tensor.matmul(
                                ps, w_sb[:, wl, j], xpair(xl, j),
                                start=(n == 0), stop=(n == len(mm) - 1),
                                perf_mode=DR,
                            )
                        tmp = ptmp.tile([128, 512], f32r, tag=f"{name}tmp")
                        mode = TMP_ENG.get(name, "act")
                        if mode == "dve" or (name == "q" and TMPQ_DVE):
                            nc.vector.tensor_copy(tmp, ps)  # PSUM -> SBUF
                        elif mode == "split":
                            nc.vector.tensor_copy(tmp[:, 0:256], ps[:, 0:256])
                            nc.scalar.copy(tmp[:, 256:512], ps[:, 256:512])
                        else:
                            nc.scalar.copy(tmp, ps)  # GPSIMD can't read PSUM
                        tmps[name] = tmp
                    return f

                def p_rope_q():
                    tmp = tmps["q"]
                    sq = ps_a.tile([128, 512], f32, tag="pa")
                    nc.tensor.matmul(sq, sperm_sb, tmp, start=True, stop=True)
                    cs = tg[0][:, 0, :]
                    sn = tg[0][:, 1, :]
                    m1 = ptmp.tile([128, 512], f32, tag="m1")
                    m2 = ptmp.tile([128, 512], f32, tag="m2")
                    nc.gpsimd.tensor_mul(m1, tmp, cs)
                    nc.vector.tensor_mul(m2, sq, sn)
                    nc.gpsimd.tensor_add(q8_sb[:, c, :], m1, m2)

                def p_rope_k():
                    tmp = tmps["k"]
                    sq = ps_a.tile([128, 512], f32, tag="pa")
                    nc.tensor.matmul(sq, sperm_sb, tmp, start=True, stop=True)
                    cs = tg[0][:, 0, :]
                    sn = tg[0][:, 1, :]
                    m1 = ptmp.tile([128, 512], f32, tag="m1")
                    k8h = k8_sb[:, b % 2, 0, s0:s0 + 512]
                    if not KRES:
                        # write fp8(k) straight into k8; sub1 stays zero
                        m2 = ptmp.tile([128, 512], f32, tag="m2")
                        nc.gpsimd.tensor_mul(m1, tmp, cs)
                        nc.vector.tensor_mul(m2, sq, sn)
                        nc.gpsimd.tensor_add(k8h, m1, m2)
                        return
                    kf = ptmp.tile([128, 512], f32, tag="kf")
                    if ROPE_EARLY_DVE and u <= ROPE_EARLY_U:
                        nc.vector.tensor_mul(m1, tmp, cs)
                    else:
                        nc.gpsimd.tensor_mul(m1, tmp, cs)
                    nc.vector.tensor_mul(kf, sq, sn)
                    nc.gpsimd.tensor_add(kf, m1, kf)
                    if K8_SWAP:
                        nc.gpsimd.tensor_copy(k8h, kf)
                        nc.vector.tensor_sub(k8_sb[:, b % 2, 1, s0:s0 + 512],
                                             kf, k8h)
                    else:
                        nc.vector.tensor_copy(k8h, kf)
                        nc.gpsimd.tensor_sub(k8_sb[:, b % 2, 1, s0:s0 + 512],
                                             kf, k8h)

                vps = [None]

                def p_vproj(sub):
                    def f():
                        if VPROJ_SPLIT:
                            vp = ps_a.tile([128, 128], f32, tag="pa", name="vp")
                            for d in range(8):
                                nc.tensor.matmul(
                                    vp,
                                    xslot(d)[:, 128 * sub:128 * (sub + 1)],
                                    wv_sb[:, d, :],
                                    start=(d == 0), stop=(d == 7),
                                )
                            nc.vector.tensor_copy(v_sb[:, 4 * u + sub, 0:64],
                                                  vp[:, 0:64])
                            nc.vector.tensor_copy(v_sb[:, 4 * u + sub, 128:192],
                                                  vp[:, 64:128])
                            return
                        if vps[0] is None:
                            vps[0] = ps_a.tile([128, 4, 128], f32, tag="pa",
                                               name="vps")
                        ss = slice(128 * sub, 128 * (sub + 1))
                        mm = [(0, 0, j) for j in range(4)] + \
                             [(0, 1, j) for j in range(4)] + \
                             [(1, 0, j) for j in range(4)]
                        for n, (wl, xl, j) in enumerate(mm):
                            nc.tensor.matmul(
                                vps[0][:, sub, :],
                                xpair(xl, j)[:, :, ss],
                                wv_sb[:, wl, j],
                                start=(n == 0), stop=(n == len(mm) - 1),
                                perf_mode=DR,
                            )
                    return f

                def p_vstore_a():
                    nc.vector.tensor_copy(v_sb[:, 4 * u:4 * u + 4, 0:64],
                                          vps[0][:, :, 0:64])

                def p_vstore_b():
                    nc.vector.tensor_copy(v_sb[:, 4 * u:4 * u + 4, 128:192],
                                          vps[0][:, :, 64:128])

                pieces = [p_load, p_proj(wq_sb, "q"), p_proj(wk_sb, "k"),
                          p_rope_q, p_vproj(0), p_vproj(1),
                          p_rope_k, p_vproj(2), p_vproj(3)]
                if not VPROJ_SPLIT:
                    pieces += [p_vstore_a, p_vstore_b]
                return pieces

            def proj_chunk(b, c, after_xt=None):
                for f in proj_pieces(b, c, after_xt):
                    f()

            def oproj_piece(b, c, s4, late=False, use_pa=False):
                """Output projection for one 128-row seq tile (emitted one
                chunk late, spread across the next chunk's tiles). With
                ``use_pa`` (final chunk: no projection running) the PSUM
                comes from the idle pa ring so the scores/exp pipeline's
                sps ring is untouched."""
                row0 = S * b + NQ * c + 128 * s4
                if use_pa and OPROJ_PA:
                    yp0 = ps_a.tile([128, 512], f32, tag="pa", name="yp0")
                    yp1 = ps_a.tile([128, 512], f32, tag="pa", name="yp1")
                    yph = [yp0, yp1]
                    for hn in range(2):
                        nc.tensor.matmul(
                            yph[hn],
                            oT_sb[:, row0:row0 + 128],
                            wo_sb[:, 512 * hn:512 * (hn + 1)],
                            start=True, stop=True,
                        )
                    ys = pys.tile([128, 1024], bf16, tag="ys")
                    if OPROJ_PA_ACT:
                        nc.scalar.copy(ys[:, 0:512], yp0)
                        nc.scalar.copy(ys[:, 512:1024], yp1)
                    else:
                        nc.vector.tensor_copy(ys[:, 0:512], yp0)
                        nc.scalar.copy(ys[:, 512:1024], yp1)
                    if OPROJ_DMA_SPLIT:
                        # first half ships while the second copy still runs
                        nc.sync.dma_start(out=y[row0:row0 + 128, 0:512],
                                          in_=ys[:, 0:512])
                        nc.sync.dma_start(out=y[row0:row0 + 128, 512:1024],
                                          in_=ys[:, 512:1024])
                    else:
                        nc.sync.dma_start(out=y[row0:row0 + 128, :], in_=ys)
                    return
                yp = ps_s.tile([128, 2, 512], f32, tag="sps")
                for hn in range(2):
                    nc.tensor.matmul(
                        yp[:, hn, :],
                        oT_sb[:, row0:row0 + 128],
                        wo_sb[:, 512 * hn:512 * (hn + 1)],
                        start=True, stop=True,
                    )
                ys = pys.tile([128, 1024], bf16, tag="ys")
                # keep ACT free of copies in the exp-bound late chunks
                if YS_MODE == 2 or (YS_MODE == 3 and late):
                    nc.vector.tensor_copy(ys[:, 0:512], yp[:, 0, :])
                    nc.scalar.copy(ys[:, 512:1024], yp[:, 1, :])
                elif YS_MODE == 0 or YS_MODE == 3:
                    if s4 % 2 == 0:
                        nc.vector.tensor_copy(ys, yp.rearrange("p a n -> p (a n)"))
                    else:
                        nc.scalar.copy(ys, yp.rearrange("p a n -> p (a n)"))
                elif late == "tail":
                    nc.scalar.copy(ys[:, 0:512], yp[:, 0, :])
                    nc.vector.tensor_copy(ys[:, 512:1024], yp[:, 1, :])
                elif late or s4 % 2 == 0:
                    nc.vector.tensor_copy(ys, yp.rearrange("p a n -> p (a n)"))
                else:
                    nc.scalar.copy(ys, yp.rearrange("p a n -> p (a n)"))
                nc.sync.dma_start(out=y[row0:row0 + 128, :], in_=ys)

            def attn_chunk(b, c, mids=(), fine_tail=False, group_tail=False,
                           tail_cb=None):
                """Causal attention for query chunk c of batch b. ``mids`` are
                emitted one per attention tile (pipelined filler work such as
                the previous chunk's output projection). With ``group_tail``
                (final chunk), PV accumulation stops per 128-query column
                group so normalize+oproj+store overlap the remaining tiles."""
                mids = list(mids)
                qsl = slice(S * b + NQ * c, S * b + NQ * (c + 1))
                nt = (NQ // NK) * (c + 1)
                oa = ps_o.tile([128, 512], f32, tag="oacc")
                ob = ps_o.tile([128, 512], f32, tag="oacc")
                pending = []  # (p tile, j, t) awaiting PV matmul
                PV_DEPTH = PV_DEPTH_N

                def finish_group(g):
                    # group g of this chunk is fully accumulated: normalize,
                    # project, store - all while later tiles still run
                    fs = slice(128 * g, 128 * (g + 1))
                    qs4 = slice(qsl.start + 128 * g, qsl.start + 128 * (g + 1))
                    rra = pr.tile([64, 128], f32, tag="rra")
                    rrb = pr.tile([64, 128], f32, tag="rrb")
                    RECIP(rra, oa[64:128, fs])
                    nc.vector.tensor_mul(oT_sb[0:64, qs4], oa[0:64, fs], rra)
                    RECIP(rrb, ob[0:64, fs])
                    nc.vector.tensor_mul(oT_sb[64:128, qs4], ob[64:128, fs], rrb)
                    oproj_piece(b, c, g, late="tail" if g == 3 else True,
                                use_pa=True)

                def pv_flush():
                    p, j, _t = pending.pop(0)
                    w0 = 128 * j
                    if group_tail:
                        for g in range(j, 4):
                            gs = slice(128 * g, 128 * (g + 1))
                            nc.tensor.matmul(
                                oa[:, gs], v_sb[:, 16 * b + _t, 0:128],
                                p[:, 0, gs],
                                start=(_t == 0), stop=(_t == 4 * c + g),
                            )
                            nc.tensor.matmul(
                                ob[:, gs], v_sb[:, 16 * b + _t, 64:192],
                                p[:, 1, gs],
                                start=(_t == 0), stop=(_t == 4 * c + g),
                            )
                        if _t >= 4 * c:
                            finish_group(_t - 4 * c)
                        return
                    segs = [(w0, 512)]
                    if MASK_SPLIT and 0 < _t >= 4 * c and w0 + 128 < 512:
                        # masked diagonal block separately, so the clean
                        # columns' PV never waits on the mask multiply
                        segs = [(w0 + 128, 512), (w0, w0 + 128)]
                    for lo_, hi_ in segs:
                        nc.tensor.matmul(
                            oa[:, lo_:hi_], v_sb[:, 16 * b + _t, 0:128],
                            p[:, 0, lo_:hi_],
                            start=(_t == 0), stop=(_t == nt - 1),
                        )
                        nc.tensor.matmul(
                            ob[:, lo_:hi_], v_sb[:, 16 * b + _t, 64:192],
                            p[:, 1, lo_:hi_],
                            start=(_t == 0), stop=(_t == nt - 1),
                        )

                for t in range(nt):
                    j = max(0, t - 4 * c)  # within-chunk diagonal offset
                    w0 = 128 * j           # causally-dead query columns
                    sps = ps_s.tile([128, 2, 512], f32, tag="sps")
                    for h in range(2):
                        hs = slice(64 * h, 64 * h + 64)
                        nc.tensor.matmul(
                            sps[:, h, w0:512],
                            k8_sb[hs, b % 2, :, 128 * t:128 * (t + 1)],
                            q8_sb[hs, c, w0:512].unsqueeze(1)
                            .to_broadcast([64, 2, 512 - w0]),
                            start=True, stop=True, perf_mode=DR,
                        )
                    p = pp.tile([128, 2, 512], bf16, tag="p")
                    nc.scalar.activation(
                        p[:, :, w0:512], sps[:, :, w0:512], EXP, scale=0.125 / 1024.0,
                    )
                    if t >= 4 * c:  # diagonal tile: mask boundary block
                        pb = p[:, :, w0:w0 + 128]
                        nc.vector.tensor_mul(
                            pb, pb, m128_sb.unsqueeze(1).to_broadcast([128, 2, 128]),
                        )
                    if len(pending) >= PV_DEPTH:
                        pv_flush()
                    pending.append((p, j, t))
                    if mids:
                        mids.pop(0)()
                while pending:
                    pv_flush()
                for m in mids:  # in case nt < len(mids)
                    m()
                if group_tail:
                    return

                # oa rows 64:128 / ob rows 0:64 hold the replicated
                # softmax denominators (from the ones block in V).
                rra = pr.tile([64, 512], f32, tag="rra")
                rrb = pr.tile([64, 512], f32, tag="rrb")
                if fine_tail:
                    # per-seq-tile normalize so the trailing output projection
                    # can start before the whole chunk is normalized
                    for s4 in range(4):
                        fs = slice(128 * s4, 128 * (s4 + 1))
                        qs4 = slice(qsl.start + 128 * s4, qsl.start + 128 * (s4 + 1))
                        RECIP(rra[:, fs], oa[64:128, fs])
                        nc.vector.tensor_mul(oT_sb[0:64, qs4], oa[0:64, fs],
                                             rra[:, fs])
                        RECIP(rrb[:, fs], ob[0:64, fs])
                        nc.vector.tensor_mul(oT_sb[64:128, qs4], ob[64:128, fs],
                                             rrb[:, fs])
                        if tail_cb is not None:
                            tail_cb(s4)
                else:
                    RECIP(rra, oa[64:128, :])
                    RECIP(rrb, ob[0:64, :])
                    eng = nc.gpsimd if NORM_POOL else nc.vector
                    eng.tensor_mul(oT_sb[0:64, qsl], oa[0:64, :], rra)
                    eng.tensor_mul(oT_sb[64:128, qsl], ob[64:128, :], rrb)

            # Software pipeline: projections run one chunk ahead of attention;
            # output projections trail their attention chunk by one.
            def oproj_mids(bc, late=False, use_pa=False):
                if bc is None:
                    return ()
                return [lambda s4=s4: oproj_piece(bc[0], bc[1], s4, late=late,
                                                  use_pa=use_pa)
                        for s4 in range(4)]

            owed = []  # (b, c, s4) oproj pieces not yet emitted
            prev = None
            for b in range(B):
                if b == 0:
                    proj_chunk(b, 0, after_xt=late_consts)
                for c in range(4):
                    # deferrable oproj queue: per-chunk quota steers this PE
                    # filler out of PE-bound chunks into the exp-bound ones
                    late_ = (b == 1 or c == 3)
                    pa_ = (b + 1 == B and c == 3)
                    quota = OPROJ_Q[4 * b + c]
                    take, owed = owed[:quota], owed[quota:]
                    mids = []
                    half_ = pa_ or (OPROJ_HALF2 and b == 0 and c == 3)
                    for t in take:
                        if half_ and OPROJ_HALF:
                            # half-granularity: one matmul+copy per pass slot
                            def mk(t=t):
                                st = {}

                                pool_, tag_ = ((ps_a, "pa") if pa_
                                               else (ps_s, "sps"))

                                def f1():
                                    row0 = S * t[0] + NQ * t[1] + 128 * t[2]
                                    yp0 = pool_.tile([128, 512], f32,
                                                     tag=tag_, name="yp0")
                                    nc.tensor.matmul(
                                        yp0, oT_sb[:, row0:row0 + 128],
                                        wo_sb[:, 0:512],
                                        start=True, stop=True)
                                    ys = pys.tile([128, 1024], bf16, tag="ys")
                                    nc.vector.tensor_copy(ys[:, 0:512], yp0)
                                    st["ys"] = ys

                                def f2():
                                    row0 = S * t[0] + NQ * t[1] + 128 * t[2]
                                    yp1 = pool_.tile([128, 512], f32,
                                                     tag=tag_, name="yp1")
                                    nc.tensor.matmul(
                                        yp1, oT_sb[:, row0:row0 + 128],
                                        wo_sb[:, 512:1024],
                                        start=True, stop=True)
                                    ys = st["ys"]
                                    nc.scalar.copy(ys[:, 512:1024], yp1)
                                    if OPROJ_DMA_SPLIT:
                                        nc.sync.dma_start(
                                            out=y[row0:row0 + 128, 0:512],
                                            in_=ys[:, 0:512])
                                        nc.sync.dma_start(
                                            out=y[row0:row0 + 128, 512:1024],
                                            in_=ys[:, 512:1024])
                                    else:
                                        nc.sync.dma_start(
                                            out=y[row0:row0 + 128, :], in_=ys)
                                return [f1, f2]
                            mids += mk()
                        else:
                            mids.append(
                                (lambda t=t: oproj_piece(
                                    t[0], t[1], t[2], late=late_,
                                    use_pa=pa_)))
                    if c + 1 < 4:
                        pieces = proj_pieces(b, c + 1)
                    elif b + 1 < B:
                        # thread the next batch's first projection through
                        # this batch's last attention chunk
                        pieces = proj_pieces(b + 1, 0)
                    else:
                        pieces = []
                    if MERGE_V2:
                        # spread filler evenly over this chunk's nt passes
                        work = []
                        while pieces or mids:
                            if pieces:
                                work.append(pieces.pop(0))
                            if mids:
                                work.append(mids.pop(0))
                        nt_ = 4 * (c + 1)
                        slots = [[] for _ in range(nt_)]
                        nw = len(work)
                        for i, w_ in enumerate(work):
                            slots[min(nt_ - 1, i * nt_ // max(nw, 1))].append(w_)
                        mids = [(lambda fs=fs: [f() for f in fs])
                                for fs in slots]
                    else:
                        merged = []
                        while pieces or mids:
                            if pieces:
                                merged.append(pieces.pop(0))
                            if mids:
                                merged.append(mids.pop(0))
                        mids = merged
                    last = b + 1 == B and c == 3
                    cb = None
                    if last and TAIL_CB:
                        cb = (lambda s4: oproj_piece(b, c, s4, late="tail",
                                                     use_pa=True))
                    attn_chunk(b, c, mids=mids, group_tail=last and GROUP_TAIL,
                               fine_tail=FINE_TAIL or (last and not GROUP_TAIL),
                               tail_cb=cb)
                    prev = (b, c)
                    if cb is None:
                        owed += [(b, c, s4) for s4 in range(4)]
            for t in owed:
                oproj_piece(t[0], t[1], t[2], late="tail", use_pa=True)

    nc.compile()
    _RT.update(
        nc=nc, run_bass_kernel_spmd=run_bass_kernel_spmd, mybir=mybir,
    )
    return _RT


def _host_inputs(q_weight, k_weight, v_weight, o_weight, in_features):
    """Build the per-core input maps (host-side sharding + layout prep)."""
    x = np.ascontiguousarray(np.asarray(in_features, dtype=np.float32))
    qw = np.asarray(q_weight, dtype=np.float32)
    kw = np.asarray(k_weight, dtype=np.float32)
    vw = np.asarray(v_weight, dtype=np.float32)
    ow = np.asarray(o_weight, dtype=np.float32)

    import ml_dtypes
    FP8 = ml_dtypes.float8_e4m3fn
    xT = np.ascontiguousarray(x.reshape(BS, D_MODEL).T)
    xh8 = xT.astype(FP8)
    xl8 = (xT - xh8.astype(np.float32)).astype(FP8)

    def w8pair(w):
        # [1024, 128] -> [2(hi/lo), 128(part), 4(pair), 2(sub), 128(col)]
        w32 = w * 32.0
        hi = w32.astype(FP8)
        lo = (w32 - hi.astype(np.float32)).astype(FP8)
        out = np.stack([hi, lo])
        # [2(hi/lo), 1024, 128] -> [128(part), 2, 4(pair), 2(sub), 128]
        out = out.reshape(2, 4, 2, 128, 128).transpose(3, 0, 1, 2, 4)
        return np.ascontiguousarray(out)

    perm64 = np.concatenate([np.arange(0, 64, 2), np.arange(1, 64, 2)])

    half = D_HEAD // 2
    inv_freq = THETA ** (-(np.arange(half, dtype=np.float64) * 2.0 / D_HEAD))
    pos = np.arange(S, dtype=np.float64)
    ang = pos[None, :] * inv_freq[:, None]        # [32, S]
    angf = np.tile(ang, (4, 1))                   # [128, S], row p -> i = p % 32
    trig = np.ascontiguousarray(np.stack(
        [np.cos(angf), np.sin(angf)], axis=1).astype(np.float32))

    spermT = np.zeros((128, 128), dtype=np.float32)
    for h in range(2):
        for i in range(32):
            spermT[h * 64 + 32 + i, h * 64 + i] = -1.0
            spermT[h * 64 + i, h * 64 + 32 + i] = 1.0

    kq = np.arange(128)
    mask128 = (np.arange(128)[None, :] >= kq[:, None]).astype(ml_dtypes.bfloat16)

    shared = dict(xh8=xh8, xl8=xl8, trig=trig, sperm=spermT, mask128=mask128)

    in_maps = []
    for c in range(N_CORES):
        rows = slice(128 * c, 128 * (c + 1))

        def permqk(w):
            wc = w[rows]
            return np.ascontiguousarray(
                np.concatenate([wc[0:64][perm64], wc[64:128][perm64]]).T)

        in_maps.append(dict(
            shared,
            wq8=w8pair(permqk(qw)),
            wk8=w8pair(permqk(kw)),
            wv8=w8pair(np.ascontiguousarray(vw[rows].T)),
            wo=np.ascontiguousarray(ow[:, rows].T) / 32.0,
        ))
    return in_maps


def kernel(q_weight, k_weight, v_weight, o_weight, in_features):
    rt = _build()
    in_maps = _host_inputs(q_weight, k_weight, v_weight, o_weight, in_features)
    res = rt["run_bass_kernel_spmd"](
        rt["nc"], in_maps, core_ids=list(range(N_CORES)),
    )
    y = np.zeros((BS, D_MODEL), dtype=np.float32)
    for c in range(N_CORES):
        y += np.asarray(res.results[c]["y"], dtype=np.float32)
    return y.reshape(B, S, D_MODEL)


# revision 67
# speedup vs baseline: 1.0010x; 1.0010x over previous
"""Trainium2 Bass kernel for multi-head attention (B=2, S=2048, D=1024, H=16, causal, RoPE).

Sharding: tensor-parallel over heads. Each of the 8 cores computes 2 heads
(128 of the 1024 q/k/v dims): QKV projections for its head slice, RoPE,
causal attention, and a partial output projection against its 128-column
slice of o_weight. The host sums the 8 partial outputs (the all-reduce).

Device-side layout choices:
  - QKV projections run as fp8e4 DoubleRow matmuls (2 contraction rows per
    partition, half cost per the PE model): x and the weights arrive as
    host-prepared fp8 hi+lo pairs (weights pre-scaled by 32 so fp8 normals
    cover them; the 32*32 factor is folded into the exp scale and into wo),
    and hi*hi + hi*lo + lo*hi compensation keeps the error ~0.1%.
  - Activations live transposed: q/k are [128 (head dims), seq] so the
    scores matmul contracts dh on partitions. RoPE pairs are de-interleaved
    on the host (weight-row permutation) so pair partners sit 32 partitions
    apart; the rotate step is a single 128x128 sign-swap matmul (sperm).
  - Scores also run as fp8e4 DoubleRow at half cost: the stationary pair is
    (fp8(k), k - fp8(k)) so the k-side quantization error cancels; the
    moving q8 is a stride-0 broadcast over the pair dim. Scores are
    computed transposed ([sk, sq]) so P = exp(scores) feeds the PV matmul
    directly as the moving bf16 operand. V carries a block of 64 ones
    columns so the PV matmul also emits the softmax denominator;
    normalization is a plain reciprocal+multiply.
  - V is projected directly into [seq, dh] layout by using the (transposed)
    x tiles as the stationary operand - no on-chip transposes needed.
  - Work is software-pipelined: projection chunks run one chunk ahead of
    attention, output projections trail one chunk behind, both threaded
    through the attention tile loop. Dummy matmuls at t=0 cover the first
    DMAs and pre-ramp the PE p-state clock. Engine assignment of the
    PSUM-evacuation copies (ACT/DVE only - GPSIMD cannot touch PSUM) and
    the scheduling knobs below were tuned by timeline-simulator sweep.
  - Only one PSUM accumulation group may live per 2KB bank (zero region),
    so accumulators are bank-aligned and column-split starts are avoided.
  - Partial outputs are written in bf16 (summed in fp32 on the host).
"""

import numpy as np

D_MODEL = 1024
N_HEADS = 16
D_HEAD = 64
THETA = 10000.0
B = 2
S = 2048
N_CORES = 8
BS = B * S  # 4096
NQ = 512    # query chunk width
NK = 128    # key tile width

GROUP_TAIL = False
FINE_TAIL = False
N_DUMMY = 13
VPROJ_SPLIT = False
YS_MODE = 1
PV_DEPTH_N = 3
MASK_SPLIT = False
TMPQ_DVE = False    # q-proj PSUM->SBUF copy on DVE (else Pool)
K8_SWAP = False     # k8 copy on DVE + kres on Pool
PLOAD_V2 = False    # trig before x_lo in chunk loads
KRES = True         # k-side scores-stage residual compensation
KXLO = True         # k-projection x_lo term
# oproj mid quota per chunk (0,0)..(1,3); leftovers drain after the loop
OPROJ_Q = [0, 2, 2, 10, 0, 0, 4, 14]
OPROJ_PA = True     # final-chunk oproj uses the idle pa PSUM ring
OPROJ_PA_ACT = False  # tail ys copies fully on ACT
TAIL_CB = True      # interleave final-chunk oproj with per-tile normalize
ROPE_EARLY_DVE = True  # early chunks: rope k-mul on DVE to unload Pool
ROPE_EARLY_U = 2
WDMA_LATE = False   # wk/wv/m128 DMAs dispatch after chunk-0 x_hi
TMP_ENG = {"q": "dve", "k": "act"}  # per-proj tmp-copy engine
OPROJ_HALF = True
OPROJ_DMA_SPLIT = False  # tail y DMAs in halves (worse: dispatch overhead)
FAST_RECIP = False  # 5x faster approx reciprocal for softmax denominators
OPROJ_HALF2 = False  # half-granularity mids also for the (0,3) chunk
MERGE_V2 = False    # spread filler evenly across the chunk's passes
NORM_POOL = False   # normalize multiplies on Pool instead of DVE
PP_BUFS = 5
PYS_BUFS = 8
PX_BUFS = 4
PTMP_BUFS = 4

_RT = {}


def _build():
    if _RT:
        return _RT
    import sys
    try:
        import concourse.bass  # noqa: F401
    except ImportError:
        sys.path.insert(0, "/opt/trn_rl_repo")
    import concourse.mybir as mybir
    import concourse.tile as tile
    from concourse import bacc
    from concourse._compat import axon_active
    from concourse.bass_utils import run_bass_kernel_spmd

    f32 = mybir.dt.float32
    f32r = mybir.dt.float32r
    bf16 = mybir.dt.bfloat16
    fp8 = mybir.dt.float8e4
    EXP = mybir.ActivationFunctionType.Exp
    DR = mybir.MatmulPerfMode.DoubleRow

    nc = bacc.Bacc(
        "TRN2", target_bir_lowering=False, debug=not axon_active(),
        num_devices=N_CORES,
    )

    # x and the qkv weights arrive as fp8 hi/lo pairs (weights pre-scaled by
    # 32 on the host so fp8 normals cover them; the 32*32 factor is folded
    # into the exp scale and wo). Same DMA bytes as bf16.
    xh8 = nc.dram_tensor("xh8", [D_MODEL, BS], fp8, kind="ExternalInput").ap()
    xl8 = nc.dram_tensor("xl8", [D_MODEL, BS], fp8, kind="ExternalInput").ap()
    wq8 = nc.dram_tensor("wq8", [128, 2, 4, 2, 128], fp8, kind="ExternalInput").ap()
    wk8 = nc.dram_tensor("wk8", [128, 2, 4, 2, 128], fp8, kind="ExternalInput").ap()
    wv8 = nc.dram_tensor("wv8", [128, 2, 4, 2, 128], fp8, kind="ExternalInput").ap()
    wo = nc.dram_tensor("wo", [128, D_MODEL], f32r, kind="ExternalInput").ap()
    trig = nc.dram_tensor("trig", [128, 2, S], f32, kind="ExternalInput").ap()
    sperm = nc.dram_tensor("sperm", [128, 128], f32r, kind="ExternalInput").ap()
    mask128 = nc.dram_tensor("mask128", [128, 128], bf16, kind="ExternalInput").ap()
    y = nc.dram_tensor("y", [BS, D_MODEL], bf16, kind="ExternalOutput").ap()

    with tile.TileContext(nc) as tc:
        with (
            tc.tile_pool(name="singles", bufs=1) as singles,
            tc.tile_pool(name="px", bufs=PX_BUFS) as px,
            tc.tile_pool(name="ptmp", bufs=PTMP_BUFS) as ptmp,
            tc.tile_pool(name="pp", bufs=PP_BUFS) as pp,
            tc.tile_pool(name="pys", bufs=PYS_BUFS) as pys,
            tc.tile_pool(name="pr", bufs=3) as pr,
            tc.tile_pool(name="ps_a", bufs=2, space="PSUM") as ps_a,
            tc.tile_pool(name="ps_s", bufs=2, space="PSUM") as ps_s,
            tc.tile_pool(name="ps_o", bufs=2, space="PSUM") as ps_o,
        ):
            wq_sb = singles.tile([128, 2, 4, 2, 128], fp8, tag="wq")
            wk_sb = singles.tile([128, 2, 4, 2, 128], fp8, tag="wk")
            wv_sb = singles.tile([128, 2, 4, 2, 128], fp8, tag="wv")
            wo_sb = singles.tile([128, D_MODEL], f32r, tag="wo")
            sperm_sb = singles.tile([128, 128], f32r, tag="sperm")
            m128_sb = singles.tile([128, 128], bf16, tag="m128")
            warm_sb = singles.tile([1, 2], f32, tag="warm")
            # q8: rope'd q in fp8, one slot per chunk-within-batch.
            q8_sb = singles.tile([128, 4, NQ], fp8, tag="q8")
            # k8: [head dims, batch parity, (fp8(k), k-fp8(k)), key pos]
            k8_sb = singles.tile([128, 2, 2, S], fp8, tag="k8")
            oT_sb = singles.tile([128, BS], f32r, tag="oT")
            # V tiles: [seq-tile partitions, 32 tiles, 192]: cols 0:64 head A,
            # 64:128 ones, 128:192 head B. Head A lhsT = cols 0:128, head B
            # lhsT = cols 64:192; the ones block replicates the denominator.
            v_sb = singles.tile([128, 32, 192], bf16, tag="v")

            junk_sb = singles.tile([128, 512], f32r, tag="junk")

            nc.scalar.dma_start(out=wq_sb, in_=wq8)
            if not WDMA_LATE:
                nc.scalar.dma_start(out=wk_sb, in_=wk8)
            nc.scalar.dma_start(out=sperm_sb, in_=sperm)
            if not WDMA_LATE:
                nc.scalar.dma_start(out=wv_sb, in_=wv8)
                nc.scalar.dma_start(out=m128_sb, in_=mask128)
            nc.vector.memset(v_sb[:, :, 64:128].bitcast(bf16), 1.0)

            if not KRES:
                nc.vector.memset(k8_sb[:, :, 1, :], 0.0)

            # warm the ACT exp table before the first real exp
            nc.vector.memset(warm_sb, 0.0)
            nc.scalar.activation(warm_sb[:, 0:1], warm_sb[:, 1:2], EXP)

            # dummy matmuls: keep PE busy during the initial DMAs and ramp
            # the p-state clock before the first real matmul arrives
            nc.gpsimd.memset(junk_sb.bitcast(f32), 0.0)
            for w in range(N_DUMMY):
                jp = ps_s.tile([128, 2, 512], f32, tag="sps")
                nc.tensor.matmul(jp[:, 0, :], junk_sb[:, 0:128], junk_sb,
                                 start=True, stop=True)

            def RECIP(out, in_):
                if FAST_RECIP:
                    nc.vector.reciprocal_approx_fast(out, in_)
                else:
                    nc.vector.reciprocal(out, in_)

            def late_consts():
                if WDMA_LATE:
                    nc.scalar.dma_start(out=wv_sb, in_=wv8)
                    nc.scalar.dma_start(out=m128_sb, in_=mask128)
                nc.sync.dma_start(out=wo_sb, in_=wo)

            def proj_pieces(b, c, after_xt=None):
                """QKV projections + rope + V transpose for seq chunk c of
                batch b (512 positions), as a list of closures that can be
                threaded through the attention tile loop."""
                u = 4 * b + c
                s0 = 512 * c
                xt = []
                tmps = {}
                tg = [None]

                def p_load():
                    # q/k projections need only x_hi, so trig loads before x_lo
                    xt.extend([None] * 4)

                    def load(lo, h):
                        xth = px.tile([128, 4, 512], fp8, tag=f"xt{lo}{h}",
                                      name="xth")
                        nc.sync.dma_start(
                            out=xth,
                            in_=(xh8 if lo == 0 else xl8)
                            [512 * h:512 * (h + 1), 512 * u:512 * (u + 1)]
                            .rearrange("(a p) n -> p a n", p=128),
                        )
                        xt[2 * lo + h] = xth
                    load(0, 0)
                    load(0, 1)
                    if WDMA_LATE and u == 0:
                        # wk dispatches after chunk-0 x_hi, before trig/x_lo
                        nc.scalar.dma_start(out=wk_sb, in_=wk8)
                    if PLOAD_V2:
                        tg[0] = ptmp.tile([128, 2, 512], f32, tag="tg",
                                          name="tg")
                        nc.sync.dma_start(out=tg[0],
                                          in_=trig[:, :, s0:s0 + 512])
                        load(1, 0)
                        load(1, 1)
                    else:
                        load(1, 0)
                        load(1, 1)
                        tg[0] = ptmp.tile([128, 2, 512], f32, tag="tg",
                                          name="tg")
                        nc.sync.dma_start(out=tg[0],
                                          in_=trig[:, :, s0:s0 + 512])
                    if after_xt is not None:
                        after_xt()

                def xpair(lo, j):
                    # [128, 2, 512] fp8: K-chunk pair (2j, 2j+1)
                    h, a = divmod(2 * j, 4)
                    return xt[2 * lo + h][:, a:a + 2, :]

                def p_proj(w_sb, name):
                    # DoubleRow fp8, hi/lo compensated:
                    # q/k = x_hi*w_hi + x_hi*w_lo + x_lo*w_hi
                    def f():
                        ps = ps_a.tile([128, 512], f32, tag="pa")
                        mm = [(0, 0, j) for j in range(4)] + \
                             [(1, 0, j) for j in range(4)]
                        if name == "q" or KXLO:
                            mm += [(0, 1, j) for j in range(4)]
                        for n, (wl, xl, j) in enumerate(mm):
                            nc.tensor.matmul(
                                ps, w_sb[:, wl, j], xpair(xl, j),
                                start=(n == 0), stop=(n == len(mm) - 1),
                                perf_mode=DR,
                            )
                        tmp = ptmp.tile([128, 512], f32r, tag=f"{name}tmp")
                        mode = TMP_ENG.get(name, "act")
                        if mode == "dve" or (name == "q" and TMPQ_DVE):
                            nc.vector.tensor_copy(tmp, ps)  # PSUM -> SBUF
                        elif mode == "split":
                            nc.vector.tensor_copy(tmp[:, 0:256], ps[:, 0:256])
                            nc.scalar.copy(tmp[:, 256:512], ps[:, 256:512])
                        else:
                            nc.scalar.copy(tmp, ps)  # GPSIMD can't read PSUM
                        tmps[name] = tmp
                    return f

                def p_rope_q():
                    tmp = tmps["q"]
                    sq = ps_a.tile([128, 512], f32, tag="pa")
                    nc.tensor.matmul(sq, sperm_sb, tmp, start=True, stop=True)
                    cs = tg[0][:, 0, :]
                    sn = tg[0][:, 1, :]
                    m1 = ptmp.tile([128, 512], f32, tag="m1")
                    m2 = ptmp.tile([128, 512], f32, tag="m2")
                    nc.gpsimd.tensor_mul(m1, tmp, cs)
                    nc.vector.tensor_mul(m2, sq, sn)
                    nc.gpsimd.tensor_add(q8_sb[:, c, :], m1, m2)

                def p_rope_k():
                    tmp = tmps["k"]
                    sq = ps_a.tile([128, 512], f32, tag="pa")
                    nc.tensor.matmul(sq, sperm_sb, tmp, start=True, stop=True)
                    cs = tg[0][:, 0, :]
                    sn = tg[0][:, 1, :]
                    m1 = ptmp.tile([128, 512], f32, tag="m1")
                    k8h = k8_sb[:, b % 2, 0, s0:s0 + 512]
                    if not KRES:
                        # write fp8(k) straight into k8; sub1 stays zero
                        m2 = ptmp.tile([128, 512], f32, tag="m2")
                        nc.gpsimd.tensor_mul(m1, tmp, cs)
                        nc.vector.tensor_mul(m2, sq, sn)
                        nc.gpsimd.tensor_add(k8h, m1, m2)
                        return
                    kf = ptmp.tile([128, 512], f32, tag="kf")
                    if ROPE_EARLY_DVE and u <= ROPE_EARLY_U:
                        nc.vector.tensor_mul(m1, tmp, cs)
                    else:
                        nc.gpsimd.tensor_mul(m1, tmp, cs)
                    nc.vector.tensor_mul(kf, sq, sn)
                    nc.gpsimd.tensor_add(kf, m1, kf)
                    if K8_SWAP:
                        nc.gpsimd.tensor_copy(k8h, kf)
                        nc.vector.tensor_sub(k8_sb[:, b % 2, 1, s0:s0 + 512],
                                             kf, k8h)
                    else:
                        nc.vector.tensor_copy(k8h, kf)
                        nc.gpsimd.tensor_sub(k8_sb[:, b % 2, 1, s0:s0 + 512],
                                             kf, k8h)

                vps = [None]

                def p_vproj(sub):
                    def f():
                        if VPROJ_SPLIT:
                            vp = ps_a.tile([128, 128], f32, tag="pa", name="vp")
                            for d in range(8):
                                nc.tensor.matmul(
                                    vp,
                                    xslot(d)[:, 128 * sub:128 * (sub + 1)],
                                    wv_sb[:, d, :],
                                    start=(d == 0), stop=(d == 7),
                                )
                            nc.vector.tensor_copy(v_sb[:, 4 * u + sub, 0:64],
                                                  vp[:, 0:64])
                            nc.vector.tensor_copy(v_sb[:, 4 * u + sub, 128:192],
                                                  vp[:, 64:128])
                            return
                        if vps[0] is None:
                            vps[0] = ps_a.tile([128, 4, 128], f32, tag="pa",
                                               name="vps")
                        ss = slice(128 * sub, 128 * (sub + 1))
                        mm = [(0, 0, j) for j in range(4)] + \
                             [(0, 1, j) for j in range(4)] + \
                             [(1, 0, j) for j in range(4)]
                        for n, (wl, xl, j) in enumerate(mm):
                            nc.tensor.matmul(
                                vps[0][:, sub, :],
                                xpair(xl, j)[:, :, ss],
                                wv_sb[:, wl, j],
                                start=(n == 0), stop=(n == len(mm) - 1),
                                perf_mode=DR,
                            )
                    return f

                def p_vstore_a():
                    nc.vector.tensor_copy(v_sb[:, 4 * u:4 * u + 4, 0:64],
                                          vps[0][:, :, 0:64])

                def p_vstore_b():
                    nc.vector.tensor_copy(v_sb[:, 4 * u:4 * u + 4, 128:192],
                                          vps[0][:, :, 64:128])

                pieces = [p_load, p_proj(wq_sb, "q"), p_proj(wk_sb, "k"),
                          p_rope_q, p_vproj(0), p_vproj(1),
                          p_rope_k, p_vproj(2), p_vproj(3)]
                if not VPROJ_SPLIT:
                    pieces += [p_vstore_a, p_vstore_b]
                return pieces

            def proj_chunk(b, c, after_xt=None):
                for f in proj_pieces(b, c, after_xt):
                    f()

            def oproj_piece(b, c, s4, late=False, use_pa=False):
                """Output projection for one 128-row seq tile (emitted one
                chunk late, spread across the next chunk's tiles). With
                ``use_pa`` (final chunk: no projection running) the PSUM
                comes from the idle pa ring so the scores/exp pipeline's
                sps ring is untouched."""
                row0 = S * b + NQ * c + 128 * s4
                if use_pa and OPROJ_PA:
                    yp0 = ps_a.tile([128, 512], f32, tag="pa", name="yp0")
                    yp1 = ps_a.tile([128, 512], f32, tag="pa", name="yp1")
                    yph = [yp0, yp1]
                    for hn in range(2):
                        nc.tensor.matmul(
                            yph[hn],
                            oT_sb[:, row0:row0 + 128],
                            wo_sb[:, 512 * hn:512 * (hn + 1)],
                            start=True, stop=True,
                        )
                    ys = pys.tile([128, 1024], bf16, tag="ys")
                    if OPROJ_PA_ACT:
                        nc.scalar.copy(ys[:, 0:512], yp0)
                        nc.scalar.copy(ys[:, 512:1024], yp1)
                    else:
                        nc.vector.tensor_copy(ys[:, 0:512], yp0)
                        nc.scalar.copy(ys[:, 512:1024], yp1)
                    if OPROJ_DMA_SPLIT:
                        # first half ships while the second copy still runs
                        nc.sync.dma_start(out=y[row0:row0 + 128, 0:512],
                                          in_=ys[:, 0:512])
                        nc.sync.dma_start(out=y[row0:row0 + 128, 512:1024],
                                          in_=ys[:, 512:1024])
                    else:
                        nc.sync.dma_start(out=y[row0:row0 + 128, :], in_=ys)
                    return
                yp = ps_s.tile([128, 2, 512], f32, tag="sps")
                for hn in range(2):
                    nc.tensor.matmul(
                        yp[:, hn, :],
                        oT_sb[:, row0:row0 + 128],
                        wo_sb[:, 512 * hn:512 * (hn + 1)],
                        start=True, stop=True,
                    )
                ys = pys.tile([128, 1024], bf16, tag="ys")
                # keep ACT free of copies in the exp-bound late chunks
                if YS_MODE == 2 or (YS_MODE == 3 and late):
                    nc.vector.tensor_copy(ys[:, 0:512], yp[:, 0, :])
                    nc.scalar.copy(ys[:, 512:1024], yp[:, 1, :])
                elif YS_MODE == 0 or YS_MODE == 3:
                    if s4 % 2 == 0:
                        nc.vector.tensor_copy(ys, yp.rearrange("p a n -> p (a n)"))
                    else:
                        nc.scalar.copy(ys, yp.rearrange("p a n -> p (a n)"))
                elif late == "tail":
                    nc.scalar.copy(ys[:, 0:512], yp[:, 0, :])
                    nc.vector.tensor_copy(ys[:, 512:1024], yp[:, 1, :])
                elif late or s4 % 2 == 0:
                    nc.vector.tensor_copy(ys, yp.rearrange("p a n -> p (a n)"))
                else:
                    nc.scalar.copy(ys, yp.rearrange("p a n -> p (a n)"))
                nc.sync.dma_start(out=y[row0:row0 + 128, :], in_=ys)

            def attn_chunk(b, c, mids=(), fine_tail=False, group_tail=False,
                           tail_cb=None):
                """Causal attention for query chunk c of batch b. ``mids`` are
                emitted one per attention tile (pipelined filler work such as
                the previous chunk's output projection). With ``group_tail``
                (final chunk), PV accumulation stops per 128-query column
                group so normalize+oproj+store overlap the remaining tiles."""
                mids = list(mids)
                qsl = slice(S * b + NQ * c, S * b + NQ * (c + 1))
                nt = (NQ // NK) * (c + 1)
                oa = ps_o.tile([128, 512], f32, tag="oacc")
                ob = ps_o.tile([128, 512], f32, tag="oacc")
                pending = []  # (p tile, j, t) awaiting PV matmul
                PV_DEPTH = PV_DEPTH_N

                def finish_group(g):
                    # group g of this chunk is fully accumulated: normalize,
                    # project, store - all while later tiles still run
                    fs = slice(128 * g, 128 * (g + 1))
                    qs4 = slice(qsl.start + 128 * g, qsl.start + 128 * (g + 1))
                    rra = pr.tile([64, 128], f32, tag="rra")
                    rrb = pr.tile([64, 128], f32, tag="rrb")
                    RECIP(rra, oa[64:128, fs])
                    nc.vector.tensor_mul(oT_sb[0:64, qs4], oa[0:64, fs], rra)
                    RECIP(rrb, ob[0:64, fs])
                    nc.vector.tensor_mul(oT_sb[64:128, qs4], ob[64:128, fs], rrb)
                    oproj_piece(b, c, g, late="tail" if g == 3 else True,
                                use_pa=True)

                def pv_flush():
                    p, j, _t = pending.pop(0)
                    w0 = 128 * j
                    if group_tail:
                        for g in range(j, 4):
                            gs = slice(128 * g, 128 * (g + 1))
                            nc.tensor.matmul(
                                oa[:, gs], v_sb[:, 16 * b + _t, 0:128],
                                p[:, 0, gs],
                                start=(_t == 0), stop=(_t == 4 * c + g),
                            )
                            nc.tensor.matmul(
                                ob[:, gs], v_sb[:, 16 * b + _t, 64:192],
                                p[:, 1, gs],
                                start=(_t == 0), stop=(_t == 4 * c + g),
                            )
                        if _t >= 4 * c:
                            finish_group(_t - 4 * c)
                        return
                    segs = [(w0, 512)]
                    if MASK_SPLIT and 0 < _t >= 4 * c and w0 + 128 < 512:
                        # masked diagonal block separately, so the clean
                        # columns' PV never waits on the mask multiply
                        segs = [(w0 + 128, 512), (w0, w0 + 128)]
                    for lo_, hi_ in segs:
                        nc.tensor.matmul(
                            oa[:, lo_:hi_], v_sb[:, 16 * b + _t, 0:128],
                            p[:, 0, lo_:hi_],
                            start=(_t == 0), stop=(_t == nt - 1),
                        )
                        nc.tensor.matmul(
                            ob[:, lo_:hi_], v_sb[:, 16 * b + _t, 64:192],
                            p[:, 1, lo_:hi_],
                            start=(_t == 0), stop=(_t == nt - 1),
                        )

                for t in range(nt):
                    j = max(0, t - 4 * c)  # within-chunk diagonal offset
                    w0 = 128 * j           # causally-dead query columns
                    sps = ps_s.tile([128, 2, 512], f32, tag="sps")
                    for h in range(2):
                        hs = slice(64 * h, 64 * h + 64)
                        nc.tensor.matmul(
                            sps[:, h, w0:512],
                            k8_sb[hs, b % 2, :, 128 * t:128 * (t + 1)],
                            q8_sb[hs, c, w0:512].unsqueeze(1)
                            .to_broadcast([64, 2, 512 - w0]),
                            start=True, stop=True, perf_mode=DR,
                        )
                    p = pp.tile([128, 2, 512], bf16, tag="p")
                    nc.scalar.activation(
                        p[:, :, w0:512], sps[:, :, w0:512], EXP, scale=0.125 / 1024.0,
                    )
                    if t >= 4 * c:  # diagonal tile: mask boundary block
                        pb = p[:, :, w0:w0 + 128]
                        nc.vector.tensor_mul(
                            pb, pb, m128_sb.unsqueeze(1).to_broadcast([128, 2, 128]),
                        )
                    if len(pending) >= PV_DEPTH:
                        pv_flush()
                    pending.append((p, j, t))
                    if mids:
                        mids.pop(0)()
                while pending:
                    pv_flush()
                for m in mids:  # in case nt < len(mids)
                    m()
                if group_tail:
                    return

                # oa rows 64:128 / ob rows 0:64 hold the replicated
                # softmax denominators (from the ones block in V).
                rra = pr.tile([64, 512], f32, tag="rra")
                rrb = pr.tile([64, 512], f32, tag="rrb")
                if fine_tail:
                    # per-seq-tile normalize so the trailing output projection
                    # can start before the whole chunk is normalized
                    for s4 in range(4):
                        fs = slice(128 * s4, 128 * (s4 + 1))
                        qs4 = slice(qsl.start + 128 * s4, qsl.start + 128 * (s4 + 1))
                        RECIP(rra[:, fs], oa[64:128, fs])
                        nc.vector.tensor_mul(oT_sb[0:64, qs4], oa[0:64, fs],
                                             rra[:, fs])
                        RECIP(rrb[:, fs], ob[0:64, fs])
                        nc.vector.tensor_mul(oT_sb[64:128, qs4], ob[64:128, fs],
                                             rrb[:, fs])
                        if tail_cb is not None:
                            tail_cb(s4)
                else:
                    RECIP(rra, oa[64:128, :])
                    RECIP(rrb, ob[0:64, :])
                    eng = nc.gpsimd if NORM_POOL else nc.vector
                    eng.tensor_mul(oT_sb[0:64, qsl], oa[0:64, :], rra)
                    eng.tensor_mul(oT_sb[64:128, qsl], ob[64:128, :], rrb)

            # Software pipeline: projections run one chunk ahead of attention;
            # output projections trail their attention chunk by one.
            def oproj_mids(bc, late=False, use_pa=False):
                if bc is None:
                    return ()
                return [lambda s4=s4: oproj_piece(bc[0], bc[1], s4, late=late,
                                                  use_pa=use_pa)
                        for s4 in range(4)]

            owed = []  # (b, c, s4) oproj pieces not yet emitted
            prev = None
            for b in range(B):
                if b == 0:
                    proj_chunk(b, 0, after_xt=late_consts)
                for c in range(4):
                    # deferrable oproj queue: per-chunk quota steers this PE
                    # filler out of PE-bound chunks into the exp-bound ones
                    late_ = (b == 1 or c == 3)
                    pa_ = (b + 1 == B and c == 3)
                    quota = OPROJ_Q[4 * b + c]
                    take, owed = owed[:quota], owed[quota:]
                    mids = []
                    half_ = pa_ or (OPROJ_HALF2 and b == 0 and c == 3)
                    for t in take:
                        if half_ and OPROJ_HALF:
                            # half-granularity: one matmul+copy per pass slot
                            def mk(t=t):
                                st = {}

                                pool_, tag_ = ((ps_a, "pa") if pa_
                                               else (ps_s, "sps"))

                                def f1():
                                    row0 = S * t[0] + NQ * t[1] + 128 * t[2]
                                    yp0 = pool_.tile([128, 512], f32,
                                                     tag=tag_, name="yp0")
                                    nc.tensor.matmul(
                                        yp0, oT_sb[:, row0:row0 + 128],
                                        wo_sb[:, 0:512],
                                        start=True, stop=True)
                                    ys = pys.tile([128, 1024], bf16, tag="ys")
                                    nc.vector.tensor_copy(ys[:, 0:512], yp0)
                                    st["ys"] = ys

                                def f2():
                                    row0 = S * t[0] + NQ * t[1] + 128 * t[2]
                                    yp1 = pool_.tile([128, 512], f32,
                                                     tag=tag_, name="yp1")
                                    nc.tensor.matmul(
                                        yp1, oT_sb[:, row0:row0 + 128],
                                        wo_sb[:, 512:1024],
                                        start=True, stop=True)
                                    ys = st["ys"]
                                    nc.scalar.copy(ys[:, 512:1024], yp1)
                                    if OPROJ_DMA_SPLIT:
                                        nc.sync.dma_start(
                                            out=y[row0:row0 + 128, 0:512],
                                            in_=ys[:, 0:512])
                                        nc.sync.dma_start(
                                            out=y[row0:row0 + 128, 512:1024],
                                            in_=ys[:, 512:1024])
                                    else:
                                        nc.sync.dma_start(
                                            out=y[row0:row0 + 128, :], in_=ys)
                                return [f1, f2]
                            mids += mk()
                        else:
                            mids.append(
                                (lambda t=t: oproj_piece(
                                    t[0], t[1], t[2], late=late_,
                                    use_pa=pa_)))
                    if c + 1 < 4:
                        pieces = proj_pieces(b, c + 1)
                    elif b + 1 < B:
                        # thread the next batch's first projection through
                        # this batch's last attention chunk
                        pieces = proj_pieces(b + 1, 0)
                    else:
                        pieces = []
                    if MERGE_V2:
                        # spread filler evenly over this chunk's nt passes
                        work = []
                        while pieces or mids:
                            if pieces:
                                work.append(pieces.pop(0))
                            if mids:
                                work.append(mids.pop(0))
                        nt_ = 4 * (c + 1)
                        slots = [[] for _ in range(nt_)]
                        nw = len(work)
                        for i, w_ in enumerate(work):
                            slots[min(nt_ - 1, i * nt_ // max(nw, 1))].append(w_)
                        mids = [(lambda fs=fs: [f() for f in fs])
                                for fs in slots]
                    else:
                        merged = []
                        while pieces or mids:
                            if pieces:
                                merged.append(pieces.pop(0))
                            if mids:
                                merged.append(mids.pop(0))
                        mids = merged
                    last = b + 1 == B and c == 3
                    cb = None
                    if last and TAIL_CB:
                        cb = (lambda s4: oproj_piece(b, c, s4, late="tail",
                                                     use_pa=True))
                    attn_chunk(b, c, mids=mids, group_tail=last and GROUP_TAIL,
                               fine_tail=FINE_TAIL or (last and not GROUP_TAIL),
                               tail_cb=cb)
                    prev = (b, c)
                    if cb is None:
                        owed += [(b, c, s4) for s4 in range(4)]
            for t in owed:
                oproj_piece(t[0], t[1], t[2], late="tail", use_pa=True)

    nc.compile()
    _RT.update(
        nc=nc, run_bass_kernel_spmd=run_bass_kernel_spmd, mybir=mybir,
    )
    return _RT


def _host_inputs(q_weight, k_weight, v_weight, o_weight, in_features):
    """Build the per-core input maps (host-side sharding + layout prep)."""
    x = np.ascontiguousarray(np.asarray(in_features, dtype=np.float32))
    qw = np.asarray(q_weight, dtype=np.float32)
    kw = np.asarray(k_weight, dtype=np.float32)
    vw = np.asarray(v_weight, dtype=np.float32)
    ow = np.asarray(o_weight, dtype=np.float32)

    import ml_dtypes
    FP8 = ml_dtypes.float8_e4m3fn
    xT = np.ascontiguousarray(x.reshape(BS, D_MODEL).T)
    xh8 = xT.astype(FP8)
    xl8 = (xT - xh8.astype(np.float32)).astype(FP8)

    def w8pair(w):
        # [1024, 128] -> [2(hi/lo), 128(part), 4(pair), 2(sub), 128(col)]
        w32 = w * 32.0
        hi = w32.astype(FP8)
        lo = (w32 - hi.astype(np.float32)).astype(FP8)
        out = np.stack([hi, lo])
        # [2(hi/lo), 1024, 128] -> [128(part), 2, 4(pair), 2(sub), 128]
        out = out.reshape(2, 4, 2, 128, 128).transpose(3, 0, 1, 2, 4)
        return np.ascontiguousarray(out)

    perm64 = np.concatenate([np.arange(0, 64, 2), np.arange(1, 64, 2)])

    half = D_HEAD // 2
    inv_freq = THETA ** (-(np.arange(half, dtype=np.float64) * 2.0 / D_HEAD))
    pos = np.arange(S, dtype=np.float64)
    ang = pos[None, :] * inv_freq[:, None]        # [32, S]
    angf = np.tile(ang, (4, 1))                   # [128, S], row p -> i = p % 32
    trig = np.ascontiguousarray(np.stack(
        [np.cos(angf), np.sin(angf)], axis=1).astype(np.float32))

    spermT = np.zeros((128, 128), dtype=np.float32)
    for h in range(2):
        for i in range(32):
            spermT[h * 64 + 32 + i, h * 64 + i] = -1.0
            spermT[h * 64 + i, h * 64 + 32 + i] = 1.0

    kq = np.arange(128)
    mask128 = (np.arange(128)[None, :] >= kq[:, None]).astype(ml_dtypes.bfloat16)

    shared = dict(xh8=xh8, xl8=xl8, trig=trig, sperm=spermT, mask128=mask128)

    in_maps = []
    for c in range(N_CORES):
        rows = slice(128 * c, 128 * (c + 1))

        def permqk(w):
            wc = w[rows]
            return np.ascontiguousarray(
                np.concatenate([wc[0:64][perm64], wc[64:128][perm64]]).T)

        in_maps.append(dict(
            shared,
            wq8=w8pair(permqk(qw)),
            wk8=w8pair(permqk(kw)),
            wv8=w8pair(np.ascontiguousarray(vw[rows].T)),
            wo=np.ascontiguousarray(ow[:, rows].T) / 32.0,
        ))
    return in_maps


def kernel(q_weight, k_weight, v_weight, o_weight, in_features):
    rt = _build()
    in_maps = _host_inputs(q_weight, k_weight, v_weight, o_weight, in_features)
    res = rt["run_bass_kernel_spmd"](
        rt["nc"], in_maps, core_ids=list(range(N_CORES)),
    )
    y = np.zeros((BS, D_MODEL), dtype=np.float32)
    for c in range(N_CORES):
        y += np.asarray(res.results[c]["y"], dtype=np.float32)
    return y.reshape(B, S, D_MODEL)
